# revision 1
# baseline (speedup 1.0000x reference)
"""GQA kernel for Trainium2: B=2, T=2048, D=2048, 16 q-heads / 4 kv-heads.

Sharding: 8 cores = (batch b in {0,1}) x (kv-head g in {0..3}). Each core owns
one kv head and its 4 query heads for one batch element; the Wo projection uses
the matching 512-row slice of Wo, and the host sums the 4 partial outputs per
batch element.

Per-core dataflow (everything in transposed [feature, token] layout so the PE
contraction dim is always the partition dim):
  phase 1: Q^T/K^T/V^T = W^T @ x^T   (accumulate over D in 16 k-tiles)
           RoPE applied on the PSUM->SBUF eviction (DVE), V transposed back to
           [token, feature] via PE transpose.
  phase 2: per q-head, per 512-token q-chunk m: S^T[k,q] = K^T_tile.T @ Q^T
           (only causal k-tiles), P^T = exp(S^T * scale) on ACT, triangular
           0/1 mask on the diagonal blocks (DVE), denominators = allones.T @
           P^T accumulated on PSUM, O^T = V_tile.T @ P^T accumulated on PSUM,
           normalization fused into the O^T eviction (multiply by reciprocal).
  phase 3: Y[tt, :] += O^T_slice.T @ Wo_slice, PSUM->SBUF copy, DMA out.

Softmax skips the max-subtraction: scores are ~N(0,1) after the 1/sqrt(d)
scale, so exp never overflows fp32 and the result matches the reference to
fp32 rounding.
"""

import numpy as np
from contextlib import ExitStack

import concourse.bacc as bacc
import concourse.bass as bass
import concourse.mybir as mybir
import concourse.tile as tile
from concourse.bass_utils import run_bass_kernel_spmd
from concourse.masks import make_identity

B = 2
T = 2048
D = 2048
HD = 128          # head dim
NQH = 4           # q heads per core
CH = 512          # token chunk (psum free size)
NCH = T // CH     # 4
KT = T // HD      # 16 k-tiles over tokens
DT = D // HD      # 16 k-tiles over model dim
SCALE = float(HD) ** -0.5
ROPE_BASE = 10000.0

f32 = mybir.dt.float32
f32r = mybir.dt.float32r


def _r(ap):
    return ap


def _build_program():
    nc = bacc.Bacc("TRN2", target_bir_lowering=False, debug=False)

    xT = nc.dram_tensor("xT", [D, T], f32r, kind="ExternalInput").ap()
    wq = nc.dram_tensor("wq", [D, NQH * HD], f32r, kind="ExternalInput").ap()
    wk = nc.dram_tensor("wk", [D, HD], f32r, kind="ExternalInput").ap()
    wv = nc.dram_tensor("wv", [D, HD], f32r, kind="ExternalInput").ap()
    wo = nc.dram_tensor("wo", [NQH * HD, D], f32r, kind="ExternalInput").ap()
    cosT = nc.dram_tensor("cosT", [HD, T], f32, kind="ExternalInput").ap()
    sinTs = nc.dram_tensor("sinTs", [HD, T], f32, kind="ExternalInput").ap()
    ones_d = nc.dram_tensor("ones_d", [HD, HD], f32r, kind="ExternalInput").ap()
    masks_d = nc.dram_tensor("masks_d", [HD, 4 * CH], f32r, kind="ExternalInput").ap()
    y = nc.dram_tensor("y", [T, D], f32, kind="ExternalOutput").ap()

    with tile.TileContext(nc) as tc, ExitStack() as ctx:
        _kernel(ctx, tc, y, xT, wq, wk, wv, wo, cosT, sinTs, ones_d, masks_d)
    nc.compile()
    return nc


def _kernel(ctx, tc, y, xT, wq, wk, wv, wo, cosT, sinTs, ones_d, masks_d):
    nc = tc.nc

    const = ctx.enter_context(tc.tile_pool(name="const", bufs=1))
    wpool = ctx.enter_context(tc.tile_pool(name="w", bufs=1))
    xpool = ctx.enter_context(tc.tile_pool(name="x", bufs=1))
    qkpool = ctx.enter_context(tc.tile_pool(name="qk", bufs=2))
    ktpool = ctx.enter_context(tc.tile_pool(name="kt", bufs=1))
    vpool = ctx.enter_context(tc.tile_pool(name="v", bufs=1))
    vtpool = ctx.enter_context(tc.tile_pool(name="vt", bufs=2))
    ptpool = ctx.enter_context(tc.tile_pool(name="pt", bufs=4))
    rpool = ctx.enter_context(tc.tile_pool(name="recip", bufs=1))
    otpool = ctx.enter_context(tc.tile_pool(name="ot", bufs=2))
    tmppool = ctx.enter_context(tc.tile_pool(name="tmp", bufs=2))
    ypool = ctx.enter_context(tc.tile_pool(name="ystage", bufs=2))

    ps1 = ctx.enter_context(tc.tile_pool(name="ps1", bufs=2, space="PSUM"))
    pss = ctx.enter_context(tc.tile_pool(name="pss", bufs=3, space="PSUM"))
    pssum = ctx.enter_context(tc.tile_pool(name="pssum", bufs=1, space="PSUM"))
    pso = ctx.enter_context(tc.tile_pool(name="pso", bufs=1, space="PSUM"))
    psy = ctx.enter_context(tc.tile_pool(name="psy", bufs=1, space="PSUM"))

    # ---- constants built on device ----
    ident = const.tile([HD, HD], f32, tag="ident", name="ident")
    make_identity(nc, ident[:])
    allones = const.tile([HD, HD], f32r, tag="ones", name="allones")
    nc.sync.dma_start(allones[:], ones_d[:])
    # causal 0/1 masks for the 4 diagonal [128, 512] blocks: valid iff
    # q_local >= 128*r + k_local (host-generated)
    masks = []
    for r in range(4):
        m = const.tile([HD, CH], f32r, tag=f"mask{r}", name=f"mask{r}")
        nc.sync.dma_start(m[:], masks_d[:, bass.ts(r, CH)])
        masks.append(m)

    # ---- resident weights / tables ----
    cos_sb = const.tile([HD, T], f32, tag="cos", name="cos_sb")
    nc.sync.dma_start(cos_sb[:], cosT[:])
    sin_sb = const.tile([HD, T], f32, tag="sin", name="sin_sb")
    nc.sync.dma_start(sin_sb[:], sinTs[:])

    wq_sb = []
    wk_sb = []
    wv_sb = []
    for t in range(DT):
        a = wpool.tile([HD, NQH * HD], f32r, tag=f"wq{t}", name=f"wq{t}")
        nc.sync.dma_start(a[:], wq[bass.ts(t, HD), :])
        wq_sb.append(a)
        b_ = wpool.tile([HD, HD], f32r, tag=f"wk{t}", name=f"wk{t}")
        nc.sync.dma_start(b_[:], wk[bass.ts(t, HD), :])
        wk_sb.append(b_)
        c = wpool.tile([HD, HD], f32r, tag=f"wv{t}", name=f"wv{t}")
        nc.sync.dma_start(c[:], wv[bass.ts(t, HD), :])
        wv_sb.append(c)
    wo_sb = []
    for kk in range(NQH):
        a = wpool.tile([HD, D], f32r, tag=f"wo{kk}", name=f"wo{kk}")
        nc.sync.dma_start(a[:], wo[bass.ts(kk, HD), :])
        wo_sb.append(a)

    v_sb = [None] * KT     # V in [token, feature] layout, 16 tiles [128,128]
    kT_t = [None] * NCH    # K^T chunks [128, 512], live for the whole kernel
    qT_t = {}              # (h, n) -> Q^T chunk tile
    oT_t = {}              # (h, n) -> normalized O^T chunk tile

    def rope_evict(dst, psum, n):
        """dst = psum * cos + rotate_half(psum) * sin  (column chunk n)."""
        sl = bass.ts(n, CH)
        t1 = tmppool.tile([HD, CH], f32, tag="ropetmp", name=f"ropetmp_{n}")
        nc.vector.tensor_mul(t1[:], psum[:], cos_sb[:, sl])
        nc.vector.tensor_mul(dst[0:64, :], psum[64:128, :], sin_sb[0:64, sl])
        nc.vector.tensor_mul(dst[64:128, :], psum[0:64, :], sin_sb[64:128, sl])
        nc.vector.tensor_add(dst[:], dst[:], t1[:])

    for n in range(NCH):
        # ---------- phase 1: project chunk n of Q^T / K^T / V^T ----------
        # m-indices 0..3 = q heads, 4 = k, 5 = v; two sweeps of 3 so only
        # 3 psum banks are held; x is streamed twice.
        xts = []
        for t in range(DT):
            xt = xpool.tile([HD, CH], f32r, tag=f"x{t}", name=f"x_{n}_{t}")
            nc.sync.dma_start(xt[:], xT[bass.ts(t, HD), bass.ts(n, CH)])
            xts.append(xt)
        for half in range(3):
            mset = [half * 2 + i for i in range(2)]
            acc = {mi: ps1.tile([HD, CH], f32, tag="ps1", name=f"ps1_{n}_{mi}")
                   for mi in mset}
            for t in range(DT):
                xt = xts[t]
                for mi in mset:
                    if mi < 4:
                        lhs = wq_sb[t][:, bass.ts(mi, HD)]
                    elif mi == 4:
                        lhs = wk_sb[t][:]
                    else:
                        lhs = wv_sb[t][:]
                    nc.tensor.matmul(
                        acc[mi][:], _r(lhs), _r(xt[:]),
                        start=(t == 0), stop=(t == DT - 1),
                    )
            for mi in mset:
                if mi < 4:
                    dst = qkpool.tile([HD, CH], f32r, tag=f"qT{mi}",
                                      name=f"qT{mi}_{n}")
                    rope_evict(dst, acc[mi], n)
                    qT_t[(mi, n)] = dst
                elif mi == 4:
                    dst = ktpool.tile([HD, CH], f32r, tag=f"kT{n}",
                                      name=f"kT{n}")
                    rope_evict(dst, acc[mi], n)
                    kT_t[n] = dst
                else:
                    vt = vtpool.tile([HD, CH], f32, tag="vT", name=f"vT_{n}")
                    nc.vector.tensor_copy(vt[:], acc[mi][:])
        # V^T chunk -> V tiles [token, feature] via PE transpose
        for lt in range(4):
            pvt = psy.tile([HD, HD], f32, tag="psy", name=f"pvt_{n}_{lt}")
            nc.tensor.transpose(pvt[:], vt[:, bass.ts(lt, HD)], ident[:])
            j = 4 * n + lt
            vtile = vpool.tile([HD, HD], f32r, tag=f"v{j}", name=f"v{j}")
            nc.vector.tensor_copy(vtile[:], pvt[:])
            v_sb[j] = vtile

        # ---------- phase 2: attention for q-chunk m == n ----------
        jmax = 4 * n + 3
        for h in range(NQH):
            qch = qT_t[(h, n)]
            acc_sum = pssum.tile([HD, CH], f32, tag="pssum",
                                 name=f"pssum_{n}_{h}")
            acc_o = pso.tile([HD, CH], f32, tag="pso", name=f"pso_{n}_{h}")
            # software pipeline: PE computes S(j+1) while ACT/DVE finish
            # exp/mask of j, so the sum/O matmuls never stall the PE.
            pending = []
            def drain_one(last):
                jp, ptp = pending.pop(0)
                nc.tensor.matmul(acc_sum[:], _r(allones[:]), _r(ptp[:]),
                                 start=(jp == 0), stop=last and not pending)
                nc.tensor.matmul(acc_o[:], _r(v_sb[jp][:]), _r(ptp[:]),
                                 start=(jp == 0), stop=last and not pending)
            for j in range(jmax + 1):
                ps = pss.tile([HD, CH], f32, tag="pss", name=f"pss_{n}_{h}_{j}")
                nc.tensor.matmul(
                    ps[:],
                    _r(kT_t[j // 4][:, bass.ts(j % 4, HD)]),
                    _r(qch[:]),
                    start=True, stop=True,
                )
                pt = ptpool.tile([HD, CH], f32r, tag="pt", name=f"pt_{n}_{h}_{j}")
                nc.scalar.activation(pt[:], ps[:],
                                     mybir.ActivationFunctionType.Exp,
                                     scale=SCALE)
                r = j - 4 * n
                if r >= 0:
                    nc.vector.tensor_mul(pt[:], pt[:], masks[r][:])
                pending.append((j, pt))
                if len(pending) > 2:
                    drain_one(False)
            while pending:
                drain_one(True)
            rec = rpool.tile([HD, CH], f32, tag="recip", name=f"rec_{n}_{h}")
            nc.vector.reciprocal(rec[:], acc_sum[:])
            ot = otpool.tile([HD, CH], f32r, tag=f"oT{h}", name=f"oT{h}_{n}")
            nc.vector.tensor_mul(ot[:], acc_o[:], rec[:])
            oT_t[(h, n)] = ot

        # ---------- phase 3: output projection for token tiles of chunk n ---
        for lt in range(4):
            tt = 4 * n + lt
            for c in range(NCH):
                pyt = psy.tile([HD, CH], f32, tag="psy", name=f"py_{tt}_{c}")
                for kk in range(NQH):
                    nc.tensor.matmul(
                        pyt[:],
                        _r(oT_t[(kk, n)][:, bass.ts(lt, HD)]),
                        _r(wo_sb[kk][:, bass.ts(c, CH)]),
                        start=(kk == 0), stop=(kk == NQH - 1),
                    )
                ys = ypool.tile([HD, CH], f32, tag="ys", name=f"ys_{tt}_{c}")
                nc.scalar.copy(ys[:], pyt[:])
                nc.sync.dma_start(y[bass.ts(tt, HD), bass.ts(c, CH)], ys[:])


_PROGRAM = None


def _get_program():
    global _PROGRAM
    if _PROGRAM is None:
        _PROGRAM = _build_program()
    return _PROGRAM


def _rope_tables():
    inv_freq = 1.0 / (ROPE_BASE ** (np.arange(0, HD, 2, dtype=np.float32) / HD))
    t = np.arange(T, dtype=np.float32)
    freqs = t[:, None] * inv_freq[None, :]
    emb = np.concatenate([freqs, freqs], axis=-1)          # [T, HD]
    cos = np.cos(emb).astype(np.float32).T.copy()          # [HD, T]
    sin = np.sin(emb).astype(np.float32).T.copy()
    sin_signed = sin.copy()
    sin_signed[0:64] = -sin_signed[0:64]
    return cos, sin_signed


def _host_masks():
    k = np.arange(HD)[:, None]
    q = np.arange(CH)[None, :]
    cols = [(q >= 128 * r + k).astype(np.float32) for r in range(4)]
    return np.ascontiguousarray(np.concatenate(cols, axis=1))


def build_in_maps(x, Wq, Wk, Wv, Wo):
    cos, sin_signed = _rope_tables()
    ones = np.ones((HD, HD), dtype=np.float32)
    maskcat = _host_masks()
    in_maps = []
    for core in range(8):
        b = core // 4
        g = core % 4
        in_maps.append({
            "xT": np.ascontiguousarray(x[b].T).astype(np.float32),
            "wq": np.ascontiguousarray(Wq[:, g * NQH * HD:(g + 1) * NQH * HD]),
            "wk": np.ascontiguousarray(Wk[:, g * HD:(g + 1) * HD]),
            "wv": np.ascontiguousarray(Wv[:, g * HD:(g + 1) * HD]),
            "wo": np.ascontiguousarray(Wo[g * NQH * HD:(g + 1) * NQH * HD, :]),
            "cosT": cos,
            "sinTs": sin_signed,
            "ones_d": ones,
            "masks_d": maskcat,
        })
    return in_maps


def kernel(x, mask, Wq, Wk, Wv, Wo):
    x = np.asarray(x)
    in_maps = build_in_maps(x, np.asarray(Wq), np.asarray(Wk),
                            np.asarray(Wv), np.asarray(Wo))

    nc = _get_program()
    res = run_bass_kernel_spmd(nc, in_maps, list(range(8))).results

    out = np.zeros((B, T, D), dtype=np.float32)
    for core in range(8):
        out[core // 4] += res[core]["y"]
    return out



# revision 4
# speedup vs baseline: 1.6820x; 1.6820x over previous
"""GQA kernel for Trainium2: B=2, T=2048, D=2048, 16 q-heads / 4 kv-heads.

Sharding: 8 cores = (batch b in {0,1}) x (kv-head g in {0..3}). Each core owns
one kv head and its 4 query heads for one batch element; the Wo projection uses
the matching 512-row slice of Wo, and the host sums the 4 partial outputs per
batch element.

v2: full-bf16 pipeline (PE streams bf16 at ~216ns per 512-col matmul vs 300ns
for f32r, LDWEIGHTS halves and FWL kicks in). All matmul operands are bf16;
PSUM accumulation stays f32. Per-core dataflow in transposed [feature, token]
layout:

  phase 1 (chunk n of 512 tokens): Q^T/K^T/V^T = W^T @ x^T, 16 k-tiles per
    output, psum evicted via ACT copy (f32->bf16) then RoPE on DVE in bf16;
    V^T transposed to V [token, feature] tiles via PE transpose.
  phase 2: per q-head pair (shares the kv head): S^T tile [k,q] = K-slice.T @
    Q^T chunk (diagonal tiles column-restricted to the causally valid range),
    P^T = exp(S^T * scale) on ACT (bf16 out), triangular mask on the diagonal
    [128,128] block via gpsimd affine_select (POOL engine), denominator and
    O^T accumulated on psum via allones- and V-tile matmuls, normalization =
    reciprocal_approx_fast (DVE) + multiply fused into the O^T eviction.
  phase 3: Y[tt, :] += O^T_slice.T @ Wo_slice, psum evicted to bf16, DMA out;
    host upcasts and sums the 4 partial Y per batch element.

Emission order interleaves ph1(n+1) between ph2(n) and ph3(n) so the PE never
waits on the softmax normalization tail.

Softmax skips the max-subtraction: scores are ~N(0,1) after the 1/sqrt(d)
scale, so exp stays in range and the result matches to bf16 precision.
"""

import numpy as np
import ml_dtypes
from contextlib import ExitStack

import concourse.bacc as bacc
import concourse.bass as bass
import concourse.mybir as mybir
import concourse.tile as tile
from concourse.bass_utils import run_bass_kernel_spmd
from concourse.masks import make_identity

B = 2
T = 2048
D = 2048
HD = 128          # head dim
NQH = 4           # q heads per core
CH = 512          # token chunk (psum free size)
NCH = T // CH     # 4
KT = T // HD      # 16 k-tiles over tokens
DT = D // HD      # 16 k-tiles over model dim
SCALE = float(HD) ** -0.5
ROPE_BASE = 10000.0

f32 = mybir.dt.float32
bf16 = mybir.dt.bfloat16
BF = ml_dtypes.bfloat16


def _build_program():
    nc = bacc.Bacc("TRN2", target_bir_lowering=False, debug=False)

    xT = nc.dram_tensor("xT", [D, T], bf16, kind="ExternalInput").ap()
    wq = nc.dram_tensor("wq", [D, NQH * HD], bf16, kind="ExternalInput").ap()
    wk = nc.dram_tensor("wk", [D, HD], bf16, kind="ExternalInput").ap()
    wv = nc.dram_tensor("wv", [D, HD], bf16, kind="ExternalInput").ap()
    wo = nc.dram_tensor("wo", [NQH * HD, D], bf16, kind="ExternalInput").ap()
    cosT = nc.dram_tensor("cosT", [HD, T], bf16, kind="ExternalInput").ap()
    sinTs = nc.dram_tensor("sinTs", [HD, T], bf16, kind="ExternalInput").ap()
    y = nc.dram_tensor("y", [T, D], bf16, kind="ExternalOutput").ap()

    with tile.TileContext(nc) as tc, ExitStack() as ctx:
        _kernel(ctx, tc, y, xT, wq, wk, wv, wo, cosT, sinTs)
    nc.compile()
    return nc


def _kernel(ctx, tc, y, xT, wq, wk, wv, wo, cosT, sinTs):
    nc = tc.nc

    const = ctx.enter_context(tc.tile_pool(name="const", bufs=1))
    wpool = ctx.enter_context(tc.tile_pool(name="w", bufs=1))
    xpool = ctx.enter_context(tc.tile_pool(name="x", bufs=2))
    qpool = ctx.enter_context(tc.tile_pool(name="q", bufs=2))
    ktpool = ctx.enter_context(tc.tile_pool(name="kt", bufs=1))
    vpool = ctx.enter_context(tc.tile_pool(name="v", bufs=1))
    vtpool = ctx.enter_context(tc.tile_pool(name="vt", bufs=2))
    rtmp = ctx.enter_context(tc.tile_pool(name="rtmp", bufs=2))
    ptpool = ctx.enter_context(tc.tile_pool(name="pt", bufs=6))
    rpool = ctx.enter_context(tc.tile_pool(name="recip", bufs=2))
    otpool = ctx.enter_context(tc.tile_pool(name="ot", bufs=2))
    ypool = ctx.enter_context(tc.tile_pool(name="ystage", bufs=3))

    # PSUM: 8 banks total.  2 for S tiles, 4 for the per-head-pair sum/O
    # accumulators, 2 shared by phase-1 projection groups / V transposes /
    # phase-3 output groups.
    psS = ctx.enter_context(tc.tile_pool(name="psS", bufs=2, space="PSUM"))
    psA = ctx.enter_context(tc.tile_pool(name="psA", bufs=1, space="PSUM"))
    psG = ctx.enter_context(tc.tile_pool(name="psG", bufs=2, space="PSUM"))

    # ---- constants built on device ----
    ident = const.tile([HD, HD], bf16, tag="ident", name="ident")
    make_identity(nc, ident[:])
    allones = const.tile([HD, HD], bf16, tag="ones", name="allones")
    nc.gpsimd.memset(allones[:], 1.0)

    # ---- resident weights / tables ----
    cos_sb = const.tile([HD, T], bf16, tag="cos", name="cos_sb")
    nc.sync.dma_start(cos_sb[:], cosT[:])
    sin_sb = const.tile([HD, T], bf16, tag="sin", name="sin_sb")
    nc.sync.dma_start(sin_sb[:], sinTs[:])

    wq_sb = []
    wk_sb = []
    wv_sb = []
    for t in range(DT):
        b_ = wpool.tile([HD, HD], bf16, tag=f"wk{t}", name=f"wk{t}")
        nc.sync.dma_start(b_[:], wk[bass.ts(t, HD), :])
        wk_sb.append(b_)
    for t in range(DT):
        a = wpool.tile([HD, NQH * HD], bf16, tag=f"wq{t}", name=f"wq{t}")
        nc.sync.dma_start(a[:], wq[bass.ts(t, HD), :])
        wq_sb.append(a)
        c = wpool.tile([HD, HD], bf16, tag=f"wv{t}", name=f"wv{t}")
        nc.sync.dma_start(c[:], wv[bass.ts(t, HD), :])
        wv_sb.append(c)
    wo_sb = []
    for kk in range(NQH):
        a = wpool.tile([HD, D], bf16, tag=f"wo{kk}", name=f"wo{kk}")
        nc.sync.dma_start(a[:], wo[bass.ts(kk, HD), :])
        wo_sb.append(a)

    v_sb = [None] * KT     # V in [token, feature] layout, 16 tiles [128,128]
    kT_t = [None] * NCH    # K^T chunks [128, 512], live for the whole kernel
    qT_t = {}              # (h, n) -> Q^T chunk tile
    oT_t = {}              # (h, n) -> normalized O^T chunk tile
    xts_cur = {}           # t -> x tile for the chunk being projected

    def rope_evict(dst, psum, n, gi):
        """dst = psum * cos + rotate_half(psum) * sin  (column chunk n)."""
        sl = bass.ts(n, CH)
        tmp = rtmp.tile([HD, CH], bf16, tag="tmp", name=f"rtmp_{n}_{gi}")
        nc.scalar.copy(tmp[:], psum[:])
        tmps = rtmp.tile([HD, CH], bf16, tag="tmps", name=f"rtmps_{n}_{gi}")
        nc.scalar.copy(tmps[0:64, :], psum[64:128, :])
        nc.scalar.copy(tmps[64:128, :], psum[0:64, :])
        t1 = rtmp.tile([HD, CH], bf16, tag="t1", name=f"rt1_{n}_{gi}")
        nc.vector.tensor_mul(t1[:], tmp[:], cos_sb[:, sl])
        nc.vector.tensor_mul(dst[:], tmps[:], sin_sb[:, sl])
        nc.vector.tensor_add(dst[:], dst[:], t1[:])

    def phase1(n):
        # prefetch x for this chunk (first call) / already prefetched
        for t in range(DT):
            if (n, t) not in x_loaded:
                xt = xpool.tile([HD, CH], bf16, tag=f"x{t}", name=f"x_{n}_{t}")
                nc.sync.dma_start(xt[:], xT[bass.ts(t, HD), bass.ts(n, CH)])
                x_loaded[(n, t)] = xt
        xts = [x_loaded[(n, t)] for t in range(DT)]
        # groups: K first (phase 2 needs it), then Q heads, then V
        vt = None
        for gi, grp in enumerate(["k", "q0", "q1", "q2", "q3", "v"]):
            acc = psG.tile([HD, CH], f32, tag="gen", name=f"p1_{n}_{grp}")
            for t in range(DT):
                if grp == "k":
                    lhs = wk_sb[t][:]
                elif grp == "v":
                    lhs = wv_sb[t][:]
                else:
                    lhs = wq_sb[t][:, bass.ts(int(grp[1]), HD)]
                nc.tensor.matmul(acc[:], lhs, xts[t][:],
                                 start=(t == 0), stop=(t == DT - 1))
            if grp == "k":
                dst = ktpool.tile([HD, CH], bf16, tag=f"kT{n}", name=f"kT{n}")
                rope_evict(dst, acc, n, gi)
                kT_t[n] = dst
            elif grp == "v":
                vt = vtpool.tile([HD, CH], bf16, tag="vt", name=f"vT_{n}")
                nc.scalar.copy(vt[:], acc[:])
            else:
                h = int(grp[1])
                dst = qpool.tile([HD, CH], bf16, tag=f"qT{h}", name=f"qT{h}_{n}")
                rope_evict(dst, acc, n, gi)
                qT_t[(h, n)] = dst
        # V^T chunk -> V tiles [token, feature] via PE transpose
        for lt in range(4):
            pvt = psG.tile([HD, HD], bf16, tag="gen", name=f"pvt_{n}_{lt}")
            nc.tensor.transpose(pvt[:], vt[:, bass.ts(lt, HD)], ident[:])
            j = 4 * n + lt
            vtile = vpool.tile([HD, HD], bf16, tag=f"v{j}", name=f"v{j}")
            nc.scalar.copy(vtile[:], pvt[:])
            v_sb[j] = vtile
        # prefetch x for chunk n+1 (lands during the rest of this chunk)
        if n + 1 < NCH:
            for t in range(DT):
                xt = xpool.tile([HD, CH], bf16, tag=f"x{t}", name=f"x_{n+1}_{t}")
                nc.sync.dma_start(xt[:], xT[bass.ts(t, HD), bass.ts(n + 1, CH)])
                x_loaded[(n + 1, t)] = xt

    def phase2(n):
        jmax = 4 * n + 3
        for half in range(2):
            hs = (2 * half, 2 * half + 1)
            acc_s = {}
            acc_o = {}
            for idx, h in enumerate(hs):
                acc_s[h] = psA.tile([HD, CH], f32, tag=f"sum{idx}",
                                    name=f"psum_{n}_{h}")
                acc_o[h] = psA.tile([HD, CH], f32, tag=f"o{idx}",
                                    name=f"pso_{n}_{h}")
            pending = []

            def drain_one():
                jp, c0p, pts = pending.pop(0)
                sl = slice(c0p, CH)
                for h in hs:
                    nc.tensor.matmul(acc_s[h][:, sl], allones[:],
                                     pts[h][:, sl],
                                     start=(jp == 0), stop=(jp == jmax))
                for h in hs:
                    nc.tensor.matmul(acc_o[h][:, sl], v_sb[jp][:],
                                     pts[h][:, sl],
                                     start=(jp == 0), stop=(jp == jmax))

            for j in range(jmax + 1):
                r = j - 4 * n
                c0 = 128 * r if r > 0 else 0
                sl = slice(c0, CH)
                pts = {}
                for h in hs:
                    ps = psS.tile([HD, CH], f32, tag="s",
                                  name=f"pss_{n}_{h}_{j}")
                    nc.tensor.matmul(ps[:, sl],
                                     kT_t[j // 4][:, bass.ts(j % 4, HD)],
                                     qT_t[(h, n)][:, sl],
                                     start=True, stop=True)
                    pt = ptpool.tile([HD, CH], bf16, tag="pt",
                                     name=f"pt_{n}_{h}_{j}")
                    nc.scalar.activation(pt[:, sl], ps[:, sl],
                                         mybir.ActivationFunctionType.Exp,
                                         scale=SCALE)
                    if r >= 0:
                        # causal mask on the diagonal [128,128] block:
                        # keep where q_local - k_local >= 0 (POOL engine)
                        dsl = slice(128 * r, 128 * r + 128)
                        nc.gpsimd.affine_select(
                            out=pt[:, dsl], in_=pt[:, dsl],
                            pattern=[[1, 128]],
                            compare_op=mybir.AluOpType.is_ge,
                            fill=0.0, base=0, channel_multiplier=-1,
                        )
                    pts[h] = pt
                pending.append((j, c0, pts))
                if len(pending) > 1:
                    drain_one()
            while pending:
                drain_one()
            for h in hs:
                rec = rpool.tile([HD, CH], f32, tag="rec", name=f"rec_{n}_{h}")
                nc.vector.reciprocal_approx_fast(rec[:], acc_s[h][:])
                ot = otpool.tile([HD, CH], bf16, tag=f"oT{h}", name=f"oT{h}_{n}")
                nc.vector.tensor_mul(ot[:], acc_o[h][:], rec[:])
                oT_t[(h, n)] = ot

    def phase3(n):
        for lt in range(4):
            tt = 4 * n + lt
            for c in range(NCH):
                pyt = psG.tile([HD, CH], f32, tag="gen", name=f"py_{tt}_{c}")
                for kk in range(NQH):
                    nc.tensor.matmul(
                        pyt[:],
                        oT_t[(kk, n)][:, bass.ts(lt, HD)],
                        wo_sb[kk][:, bass.ts(c, CH)],
                        start=(kk == 0), stop=(kk == NQH - 1),
                    )
                ys = ypool.tile([HD, CH], bf16, tag="ys", name=f"ys_{tt}_{c}")
                nc.vector.tensor_copy(ys[:], pyt[:])
                nc.sync.dma_start(y[bass.ts(tt, HD), bass.ts(c, CH)], ys[:])

    x_loaded = {}
    phase1(0)
    phase2(0)
    for n in range(1, NCH):
        phase1(n)
        phase3(n - 1)
        phase2(n)
    phase3(NCH - 1)


_PROGRAM = None


def _get_program():
    global _PROGRAM
    if _PROGRAM is None:
        _PROGRAM = _build_program()
    return _PROGRAM


def _rope_tables():
    inv_freq = 1.0 / (ROPE_BASE ** (np.arange(0, HD, 2, dtype=np.float32) / HD))
    t = np.arange(T, dtype=np.float32)
    freqs = t[:, None] * inv_freq[None, :]
    emb = np.concatenate([freqs, freqs], axis=-1)          # [T, HD]
    cos = np.cos(emb).astype(np.float32).T.copy()          # [HD, T]
    sin = np.sin(emb).astype(np.float32).T.copy()
    sin_signed = sin.copy()
    sin_signed[0:64] = -sin_signed[0:64]
    return cos, sin_signed


def build_in_maps(x, Wq, Wk, Wv, Wo):
    cos, sin_signed = _rope_tables()
    cos = cos.astype(BF)
    sin_signed = sin_signed.astype(BF)
    in_maps = []
    for core in range(8):
        b = core // 4
        g = core % 4
        in_maps.append({
            "xT": np.ascontiguousarray(x[b].T).astype(BF),
            "wq": np.ascontiguousarray(
                Wq[:, g * NQH * HD:(g + 1) * NQH * HD]).astype(BF),
            "wk": np.ascontiguousarray(Wk[:, g * HD:(g + 1) * HD]).astype(BF),
            "wv": np.ascontiguousarray(Wv[:, g * HD:(g + 1) * HD]).astype(BF),
            "wo": np.ascontiguousarray(
                Wo[g * NQH * HD:(g + 1) * NQH * HD, :]).astype(BF),
            "cosT": cos,
            "sinTs": sin_signed,
        })
    return in_maps


def kernel(x, mask, Wq, Wk, Wv, Wo):
    x = np.asarray(x)
    in_maps = build_in_maps(x, np.asarray(Wq), np.asarray(Wk),
                            np.asarray(Wv), np.asarray(Wo))

    nc = _get_program()
    res = run_bass_kernel_spmd(nc, in_maps, list(range(8))).results

    out = np.zeros((B, T, D), dtype=np.float32)
    for core in range(8):
        out[core // 4] += np.asarray(res[core]["y"]).astype(np.float32)
    return out
